# revision 12
# baseline (speedup 1.0000x reference)
"""ChebyKANLinear Trainium2 kernel (fp16 pipeline).

Math: y[b,o] = (1/I) * sum_{i,d} T_d(c[b,i]) * W[i,o,d],  c = tanh(x)
with Chebyshev T_0=1, T_1=c, T_2=2c^2-1, T_3=4c^3-3c.
Monomial re-expression (exact linear recombination, folded on the host):
    y = (bias_u + c @ V1 + c^2 @ V2 + c^3 @ V3) / I
    V1 = W1 - 3*W3, V2 = 2*W2, V3 = 4*W3, bias_u[o] = sum_i (W0 - W2)[i,o]
V is deliberately NOT pre-divided by I: the unscaled values (std ~3e-3) sit
comfortably in fp16 normal range (V/I ~1e-5 would be subnormal), and the
1/I rides the final fused PSUM->SBUF op: y = (acc + bias_u) * (1/I).

Everything 16-bit where the 2e-2 rel-err budget allows (measured rel err
~8.6e-4 on HW): x, c, c^2, c^3, V, bias and the y output travel as fp16
(host converts y back to fp32); only PSUM accumulation is fp32. This
halves HBM<->SBUF traffic vs fp32 and runs each matmul as ONE PE pass
(fp32 needed a LOW+HIGH pair).

Sharding: 2D - batch into 4 shards x output_dim into 2 shards across the 8
NeuronCores. Per core the matmuls are computed TRANSPOSED,
    yT[o, b] = sum_k  V_k[i, o].T @ (c^k)[i, b]
so each core runs 6 fp16 matmuls of [K=128, M=128, N=512].

Perf notes baked in from trace analysis:
- THREE independent DMA paths, ONE descriptor each (a second descriptor on
  the same queue measured +0.7us serial overhead): xt0 on the sync HWDGE
  queue (fast, ~0.6us to first byte), xt1 on the scalar HWDGE queue
  (~1.5us to first byte - the ih=1 chain is consumed later), vb+bias on
  the GpSimd SWDGE path (~1us first byte, issued at body start while
  GpSimd is otherwise idle).
- tanh runs as TWO whole-tile ACTIVATEs (c0, c1): each extra chunk costs
  ~300ns fixed ACT overhead which delays the serial ACT spine.
- All squares/cubes on DVE (GpSimd tensor_tensor measured 2.6x slower
  plus a 0.4us library load).
- FIVE warmup matmuls on memset tiles bridge the PE from body start to
  the real chain so the HAM clock-gate (1.2 -> 2.4 GHz) is open; any PE
  idle gap resets the busy-window progress (measured cold 585ns vs warm
  ~380ns per N=512 pass).
- All 6 matmuls accumulate into ONE PSUM bank (same-bank back-to-back
  accumulation measured at full pass rate); the epilogue runs the two
  halves in PARALLEL: half 0 y=(acc+bias_u)*(1/I) via DVE tensor_scalar,
  half 1 via ACT Identity (acc*(1/I)+bias_s), each followed by its
  half-store on its own HWDGE queue.
- NO keep-warm dummy stores: a tiny DRAM write has ~3.5us completion
  latency and its completion semaphore sits ahead of the y stores in the
  queue FIFO, delaying the measured end by far more than the ~0.5us
  queue re-startup it was meant to hide (measured both ways).
"""

from contextlib import ExitStack

import numpy as np

import concourse.bass as bass
import concourse.tile as tile
from concourse import bacc, mybir
from concourse.bass_utils import run_bass_kernel_spmd

N_CORES = 8
B, I, O, D = 2048, 256, 256, 4
RB, SO = 4, 2  # batch shards x output shards
BL = B // RB  # 512 batch rows per core
OL = O // SO  # 128 output cols per core
F16 = mybir.dt.float16
F32 = mybir.dt.float32
INV_I = 1.0 / I

_cache = {}


def _build_program():
    nc = bacc.Bacc("TRN2", target_bir_lowering=False, debug=False, num_devices=N_CORES)

    # xt{ih}[i, b] = x[b, ih*128 + i]  (pre-transposed on host)
    xt0_d = nc.dram_tensor("xt0", [128, BL], F16, kind="ExternalInput")
    xt1_d = nc.dram_tensor("xt1", [128, BL], F16, kind="ExternalInput")
    # packed V: col d*OL + o holds V[d, :, o] per ih block; col 768 = bias_u
    vb_d = nc.dram_tensor("vb", [128, 6 * OL + 1], F16, kind="ExternalInput")
    # transposed fp16 output [o_local, b_local]
    y_d = nc.dram_tensor("y", [OL, BL], F16, kind="ExternalOutput")

    with tile.TileContext(nc) as tc, ExitStack() as ctx:
        pool = ctx.enter_context(tc.tile_pool(name="main", bufs=1))
        psum = ctx.enter_context(
            tc.tile_pool(name="psum", bufs=1, space=bass.MemorySpace.PSUM)
        )

        # PE warmup operands (DVE is idle this early; values are irrelevant)
        wu_w = pool.tile([128, 128], F16, tag="wu_w")
        nc.vector.memset(wu_w[:], 1.0)
        wu_r = pool.tile([128, 512], F16, tag="wu_r")
        nc.vector.memset(wu_r[:], 1.0)

        # Three parallel input paths, one descriptor each.
        xt = {}
        for ih, (eng, xd) in enumerate(((nc.sync, xt0_d), (nc.scalar, xt1_d))):
            xt[ih] = pool.tile([128, BL], F16, tag=f"xt{ih}", name=f"xt{ih}")
            eng.dma_start(xt[ih][:], xd[:])
        vb = pool.tile([128, 6 * OL + 1], F16, tag="vb")
        nc.gpsimd.dma_start(vb[:], vb_d[:])

        # Warmup matmul bridge: the HAM clock-gate needs ~3.4us of sustained
        # PE busy (free-running 4096-cycle window) before it opens 2.4 GHz,
        # so start the PE AS EARLY AS POSSIBLE: two tiny N=128 matmuls that
        # need only the first (small) memset, then N=512 ones once wu_r is
        # set. Short sub-us gaps don't re-throttle (going cold needs ~3.4us
        # of idle), so bridge length is not precision-critical.
        wu_acc = psum.tile([128, 512], F32, tag="wu_acc")
        for _ in range(2):
            nc.tensor.matmul(
                wu_acc[:, :128], wu_w[:], wu_w[:], start=True, stop=True
            )
        for _ in range(5):
            nc.tensor.matmul(wu_acc[:], wu_w[:], wu_r[:], start=True, stop=True)
        nc.tensor.matmul(
            wu_acc[:, :256], wu_w[:], wu_r[:, :256], start=True, stop=True
        )

        # tensor_scalar's ptr operand must be fp32: upconvert the fp16 bias
        # column on GpSimd right after vb lands (off the critical path).
        bias_col = pool.tile([128, 1], F32, tag="bias32")
        nc.gpsimd.tensor_scalar(
            bias_col[:],
            vb[:, 6 * OL : 6 * OL + 1],
            0.0,
            None,
            mybir.AluOpType.add,
        )
        # basis: c = tanh(xT) on ACT (one whole-tile op per half);
        # squares/cubes on DVE, ordered ih0 first (its inputs land first).
        basis = {}
        for ih in range(2):
            c = pool.tile([128, BL], F16, tag=f"c{ih}", name=f"c{ih}")
            nc.scalar.activation(c[:], xt[ih][:], mybir.ActivationFunctionType.Tanh)
            basis[(0, ih)] = c
        for ih in range(2):
            c2 = pool.tile([128, BL], F16, tag=f"c2{ih}", name=f"c2{ih}")
            nc.vector.tensor_mul(c2[:], basis[(0, ih)][:], basis[(0, ih)][:])
            basis[(1, ih)] = c2
            c3 = pool.tile([128, BL], F16, tag=f"c3{ih}", name=f"c3{ih}")
            nc.vector.tensor_mul(c3[:], c2[:], basis[(0, ih)][:])
            basis[(2, ih)] = c3

        # yT[o, b] accumulation: 6 single-pass fp16 matmuls into ONE PSUM
        # bank, ordered by operand readiness.
        acc = psum.tile([128, BL], F32, tag="acc")
        mm_order = [(0, 0), (1, 0), (2, 0), (0, 1), (1, 1), (2, 1)]
        for n, (d, ih) in enumerate(mm_order):
            col = (ih * 3 + d) * OL
            nc.tensor.matmul(
                acc[:OL, :],
                vb[:, col : col + OL],
                basis[(d, ih)][:],
                start=(n == 0),
                stop=(n == len(mm_order) - 1),
            )

        # Fused epilogue, two DVE halves: y = (acc + bias_u) * (1/I) -> fp16
        # SBUF, each half-store on its own HWDGE queue as soon as its half
        # is ready. (A second PSUM reader on another engine serializes
        # behind the first anyway - measured - so one engine loses nothing.)
        hb = BL // 2
        y_sb0 = pool.tile([OL, hb], F16, tag="y_sb0")
        y_sb1 = pool.tile([OL, hb], F16, tag="y_sb1")
        for k, (ysb, dma_eng) in enumerate(((y_sb0, nc.sync), (y_sb1, nc.scalar))):
            nc.vector.tensor_scalar(
                ysb[:],
                acc[:OL, k * hb : (k + 1) * hb],
                bias_col[:],
                INV_I,
                mybir.AluOpType.add,
                mybir.AluOpType.mult,
            )
            dma_eng.dma_start(y_d[:, k * hb : (k + 1) * hb], ysb[:])

    nc.compile()
    return nc


def _get_program():
    if "nc" not in _cache:
        _cache["nc"] = _build_program()
    return _cache["nc"]


def _make_in_maps(x, cheby_coeffs):
    x = np.ascontiguousarray(x, dtype=np.float32)
    W = np.ascontiguousarray(cheby_coeffs, dtype=np.float32)
    assert x.shape == (B, I) and W.shape == (I, O, D)

    V = np.stack(
        [
            W[:, :, 1] - 3.0 * W[:, :, 3],
            2.0 * W[:, :, 2],
            4.0 * W[:, :, 3],
        ]
    ).astype(np.float16)  # [3, I, O] unscaled
    bias_u = (W[:, :, 0] - W[:, :, 2]).sum(axis=0, dtype=np.float32)  # [O] unscaled

    xt_shards = []
    for rb in range(RB):
        xs = x[rb * BL : (rb + 1) * BL, :].T.astype(np.float16)  # [I, BL]
        xt_shards.append(
            (np.ascontiguousarray(xs[:128]), np.ascontiguousarray(xs[128:]))
        )
    vb_shards = []
    for so in range(SO):
        vb = np.empty((128, 6 * OL + 1), dtype=np.float16)
        for ih in range(2):
            for d in range(3):
                col = (ih * 3 + d) * OL
                vb[:, col : col + OL] = V[
                    d, ih * 128 : (ih + 1) * 128, so * OL : (so + 1) * OL
                ]
        vb[:, 6 * OL] = bias_u[so * OL : (so + 1) * OL].astype(np.float16)
        vb_shards.append(vb)
    in_maps = []
    for c_id in range(N_CORES):
        rb, so = divmod(c_id, SO)
        in_maps.append(
            {
                "xt0": xt_shards[rb][0],
                "xt1": xt_shards[rb][1],
                "vb": vb_shards[so],
            }
        )
    return in_maps


def kernel(x, cheby_coeffs):
    nc = _get_program()
    in_maps = _make_in_maps(x, cheby_coeffs)
    res = run_bass_kernel_spmd(nc, in_maps, list(range(N_CORES)))
    y = np.empty((B, O), dtype=np.float32)
    for c_id in range(N_CORES):
        rb, so = divmod(c_id, SO)
        y[rb * BL : (rb + 1) * BL, so * OL : (so + 1) * OL] = (
            res.results[c_id]["y"].T.astype(np.float32)
        )
    return y


# revision 13
# speedup vs baseline: 1.0291x; 1.0291x over previous
"""ChebyKANLinear Trainium2 kernel (fp16 pipeline).

Math: y[b,o] = (1/I) * sum_{i,d} T_d(c[b,i]) * W[i,o,d],  c = tanh(x)
with Chebyshev T_0=1, T_1=c, T_2=2c^2-1, T_3=4c^3-3c.
Monomial re-expression (exact linear recombination, folded on the host):
    y = (bias_u + c @ V1 + c^2 @ V2 + c^3 @ V3) / I
    V1 = W1 - 3*W3, V2 = 2*W2, V3 = 4*W3, bias_u[o] = sum_i (W0 - W2)[i,o]
V is deliberately NOT pre-divided by I: the unscaled values (std ~3e-3) sit
comfortably in fp16 normal range (V/I ~1e-5 would be subnormal), and the
1/I rides the final fused PSUM->SBUF op: y = (acc + bias_u) * (1/I).

Everything 16-bit where the 2e-2 rel-err budget allows (measured rel err
~8.6e-4 on HW): x, c, c^2, c^3, V, bias and the y output travel as fp16
(host converts y back to fp32); only PSUM accumulation is fp32. This
halves HBM<->SBUF traffic vs fp32 and runs each matmul as ONE PE pass
(fp32 needed a LOW+HIGH pair).

Sharding: 2D - batch into 4 shards x output_dim into 2 shards across the 8
NeuronCores. Per core the matmuls are computed TRANSPOSED,
    yT[o, b] = sum_k  V_k[i, o].T @ (c^k)[i, b]
so each core runs 6 fp16 matmuls of [K=128, M=128, N=512].

Perf notes baked in from trace analysis:
- THREE independent DMA paths, ONE descriptor each (a second descriptor on
  the same queue measured +0.7us serial overhead): xt0 on the sync HWDGE
  queue (fast, ~0.6us to first byte), xt1 on the scalar HWDGE queue
  (~1.5us to first byte - the ih=1 chain is consumed later), vb+bias on
  the GpSimd SWDGE path (~1us first byte, issued at body start while
  GpSimd is otherwise idle).
- tanh runs as TWO whole-tile ACTIVATEs (c0, c1): each extra chunk costs
  ~300ns fixed ACT overhead which delays the serial ACT spine.
- All squares/cubes on DVE (GpSimd tensor_tensor measured 2.6x slower
  plus a 0.4us library load).
- FIVE warmup matmuls on memset tiles bridge the PE from body start to
  the real chain so the HAM clock-gate (1.2 -> 2.4 GHz) is open; any PE
  idle gap resets the busy-window progress (measured cold 585ns vs warm
  ~380ns per N=512 pass).
- All 6 matmuls accumulate into ONE PSUM bank (same-bank back-to-back
  accumulation measured at full pass rate); the epilogue runs the two
  halves in PARALLEL: half 0 y=(acc+bias_u)*(1/I) via DVE tensor_scalar,
  half 1 via ACT Identity (acc*(1/I)+bias_s), each followed by its
  half-store on its own HWDGE queue.
- NO keep-warm dummy stores: a tiny DRAM write has ~3.5us completion
  latency and its completion semaphore sits ahead of the y stores in the
  queue FIFO, delaying the measured end by far more than the ~0.5us
  queue re-startup it was meant to hide (measured both ways).
"""

from contextlib import ExitStack

import numpy as np

import concourse.bass as bass
import concourse.tile as tile
from concourse import bacc, mybir
from concourse.bass_utils import run_bass_kernel_spmd

N_CORES = 8
B, I, O, D = 2048, 256, 256, 4
RB, SO = 4, 2  # batch shards x output shards
BL = B // RB  # 512 batch rows per core
OL = O // SO  # 128 output cols per core
F16 = mybir.dt.float16
F32 = mybir.dt.float32
INV_I = 1.0 / I

_cache = {}


def _build_program():
    nc = bacc.Bacc("TRN2", target_bir_lowering=False, debug=False, num_devices=N_CORES)

    # xt{ih}[i, b] = x[b, ih*128 + i]  (pre-transposed on host)
    xt0_d = nc.dram_tensor("xt0", [128, BL], F16, kind="ExternalInput")
    xt1_d = nc.dram_tensor("xt1", [128, BL], F16, kind="ExternalInput")
    # packed V: col d*OL + o holds V[d, :, o] per ih block; col 768 = bias_u
    vb_d = nc.dram_tensor("vb", [128, 6 * OL + 1], F16, kind="ExternalInput")
    # transposed fp16 output [o_local, b_local]
    y_d = nc.dram_tensor("y", [OL, BL], F16, kind="ExternalOutput")

    with tile.TileContext(nc) as tc, ExitStack() as ctx:
        pool = ctx.enter_context(tc.tile_pool(name="main", bufs=1))
        psum = ctx.enter_context(
            tc.tile_pool(name="psum", bufs=1, space=bass.MemorySpace.PSUM)
        )

        # PE warmup operands (DVE is idle this early; values are irrelevant)
        wu_w = pool.tile([128, 128], F16, tag="wu_w")
        nc.vector.memset(wu_w[:], 1.0)
        wu_r = pool.tile([128, 512], F16, tag="wu_r")
        nc.vector.memset(wu_r[:], 1.0)

        # Three parallel input paths, one descriptor each.
        xt = {}
        for ih, (eng, xd) in enumerate(((nc.sync, xt0_d), (nc.scalar, xt1_d))):
            xt[ih] = pool.tile([128, BL], F16, tag=f"xt{ih}", name=f"xt{ih}")
            eng.dma_start(xt[ih][:], xd[:])
        vb = pool.tile([128, 6 * OL + 1], F16, tag="vb")
        nc.gpsimd.dma_start(vb[:], vb_d[:])

        # Warmup matmul bridge: the HAM clock-gate needs ~3.4us of sustained
        # PE busy (free-running 4096-cycle window) before it opens 2.4 GHz,
        # so start the PE AS EARLY AS POSSIBLE: two tiny N=128 matmuls that
        # need only the first (small) memset, then N=512 ones once wu_r is
        # set. Short sub-us gaps don't re-throttle (going cold needs ~3.4us
        # of idle), so bridge length is not precision-critical.
        wu_acc = psum.tile([128, 512], F32, tag="wu_acc")
        for _ in range(2):
            nc.tensor.matmul(
                wu_acc[:, :128], wu_w[:], wu_w[:], start=True, stop=True
            )
        for _ in range(5):
            nc.tensor.matmul(wu_acc[:], wu_w[:], wu_r[:], start=True, stop=True)
        nc.tensor.matmul(
            wu_acc[:, :256], wu_w[:], wu_r[:, :256], start=True, stop=True
        )

        # tensor_scalar's ptr operand must be fp32: upconvert the fp16 bias
        # column on GpSimd right after vb lands (off the critical path).
        bias_col = pool.tile([128, 1], F32, tag="bias32")
        nc.gpsimd.tensor_scalar(
            bias_col[:],
            vb[:, 6 * OL : 6 * OL + 1],
            0.0,
            None,
            mybir.AluOpType.add,
        )
        # basis: c = tanh(xT) on ACT (one whole-tile op per half);
        # squares/cubes on DVE, ordered ih0 first (its inputs land first).
        basis = {}
        for ih in range(2):
            c = pool.tile([128, BL], F16, tag=f"c{ih}", name=f"c{ih}")
            nc.scalar.activation(c[:], xt[ih][:], mybir.ActivationFunctionType.Tanh)
            basis[(0, ih)] = c
        for ih in range(2):
            c2 = pool.tile([128, BL], F16, tag=f"c2{ih}", name=f"c2{ih}")
            nc.vector.tensor_mul(c2[:], basis[(0, ih)][:], basis[(0, ih)][:])
            basis[(1, ih)] = c2
            c3 = pool.tile([128, BL], F16, tag=f"c3{ih}", name=f"c3{ih}")
            nc.vector.tensor_mul(c3[:], c2[:], basis[(0, ih)][:])
            basis[(2, ih)] = c3

        # yT[o, b] accumulation: 6 single-pass fp16 matmuls into ONE PSUM
        # bank, ordered by operand readiness.
        acc = psum.tile([128, BL], F32, tag="acc")
        mm_order = [(0, 0), (1, 0), (0, 1), (2, 0), (1, 1), (2, 1)]
        for n, (d, ih) in enumerate(mm_order):
            col = (ih * 3 + d) * OL
            nc.tensor.matmul(
                acc[:OL, :],
                vb[:, col : col + OL],
                basis[(d, ih)][:],
                start=(n == 0),
                stop=(n == len(mm_order) - 1),
            )

        # Fused epilogue, two DVE halves: y = (acc + bias_u) * (1/I) -> fp16
        # SBUF, each half-store on its own HWDGE queue as soon as its half
        # is ready. (A second PSUM reader on another engine serializes
        # behind the first anyway - measured - so one engine loses nothing.)
        hb = BL // 2
        y_sb0 = pool.tile([OL, hb], F16, tag="y_sb0")
        y_sb1 = pool.tile([OL, hb], F16, tag="y_sb1")
        for k, (ysb, dma_eng) in enumerate(((y_sb0, nc.sync), (y_sb1, nc.scalar))):
            nc.vector.tensor_scalar(
                ysb[:],
                acc[:OL, k * hb : (k + 1) * hb],
                bias_col[:],
                INV_I,
                mybir.AluOpType.add,
                mybir.AluOpType.mult,
            )
            dma_eng.dma_start(y_d[:, k * hb : (k + 1) * hb], ysb[:])

    nc.compile()
    return nc


def _get_program():
    if "nc" not in _cache:
        _cache["nc"] = _build_program()
    return _cache["nc"]


def _make_in_maps(x, cheby_coeffs):
    x = np.ascontiguousarray(x, dtype=np.float32)
    W = np.ascontiguousarray(cheby_coeffs, dtype=np.float32)
    assert x.shape == (B, I) and W.shape == (I, O, D)

    V = np.stack(
        [
            W[:, :, 1] - 3.0 * W[:, :, 3],
            2.0 * W[:, :, 2],
            4.0 * W[:, :, 3],
        ]
    ).astype(np.float16)  # [3, I, O] unscaled
    bias_u = (W[:, :, 0] - W[:, :, 2]).sum(axis=0, dtype=np.float32)  # [O] unscaled

    xt_shards = []
    for rb in range(RB):
        xs = x[rb * BL : (rb + 1) * BL, :].T.astype(np.float16)  # [I, BL]
        xt_shards.append(
            (np.ascontiguousarray(xs[:128]), np.ascontiguousarray(xs[128:]))
        )
    vb_shards = []
    for so in range(SO):
        vb = np.empty((128, 6 * OL + 1), dtype=np.float16)
        for ih in range(2):
            for d in range(3):
                col = (ih * 3 + d) * OL
                vb[:, col : col + OL] = V[
                    d, ih * 128 : (ih + 1) * 128, so * OL : (so + 1) * OL
                ]
        vb[:, 6 * OL] = bias_u[so * OL : (so + 1) * OL].astype(np.float16)
        vb_shards.append(vb)
    in_maps = []
    for c_id in range(N_CORES):
        rb, so = divmod(c_id, SO)
        in_maps.append(
            {
                "xt0": xt_shards[rb][0],
                "xt1": xt_shards[rb][1],
                "vb": vb_shards[so],
            }
        )
    return in_maps


def kernel(x, cheby_coeffs):
    nc = _get_program()
    in_maps = _make_in_maps(x, cheby_coeffs)
    res = run_bass_kernel_spmd(nc, in_maps, list(range(N_CORES)))
    y = np.empty((B, O), dtype=np.float32)
    for c_id in range(N_CORES):
        rb, so = divmod(c_id, SO)
        y[rb * BL : (rb + 1) * BL, so * OL : (so + 1) * OL] = (
            res.results[c_id]["y"].T.astype(np.float32)
        )
    return y


# revision 14
# speedup vs baseline: 1.0514x; 1.0216x over previous
"""ChebyKANLinear Trainium2 kernel (fp16 pipeline).

Math: y[b,o] = (1/I) * sum_{i,d} T_d(c[b,i]) * W[i,o,d],  c = tanh(x)
with Chebyshev T_0=1, T_1=c, T_2=2c^2-1, T_3=4c^3-3c.
Monomial re-expression (exact linear recombination, folded on the host):
    y = (bias_u + c @ V1 + c^2 @ V2 + c^3 @ V3) / I
    V1 = W1 - 3*W3, V2 = 2*W2, V3 = 4*W3, bias_u[o] = sum_i (W0 - W2)[i,o]
V is deliberately NOT pre-divided by I: the unscaled values (std ~3e-3) sit
comfortably in fp16 normal range (V/I ~1e-5 would be subnormal), and the
1/I rides the final fused PSUM->SBUF op: y = (acc + bias_u) * (1/I).

Everything 16-bit where the 2e-2 rel-err budget allows (measured rel err
~8.6e-4 on HW): x, c, c^2, c^3, V, bias and the y output travel as fp16
(host converts y back to fp32); only PSUM accumulation is fp32. This
halves HBM<->SBUF traffic vs fp32 and runs each matmul as ONE PE pass
(fp32 needed a LOW+HIGH pair).

Sharding: 2D - batch into 4 shards x output_dim into 2 shards across the 8
NeuronCores. Per core the matmuls are computed TRANSPOSED,
    yT[o, b] = sum_k  V_k[i, o].T @ (c^k)[i, b]
so each core runs 6 fp16 matmuls of [K=128, M=128, N=512].

Perf notes baked in from trace analysis:
- THREE independent DMA paths, ONE descriptor each (a second descriptor on
  the same queue measured +0.7us serial overhead): xt0 on the sync HWDGE
  queue (fast, ~0.6us to first byte), xt1 on the scalar HWDGE queue
  (~1.5us to first byte - the ih=1 chain is consumed later), vb+bias on
  the GpSimd SWDGE path (~1us first byte, issued at body start while
  GpSimd is otherwise idle).
- tanh runs as TWO whole-tile ACTIVATEs (c0, c1): each extra chunk costs
  ~300ns fixed ACT overhead which delays the serial ACT spine.
- All squares/cubes on DVE (GpSimd tensor_tensor measured 2.6x slower
  plus a 0.4us library load).
- FIVE warmup matmuls on memset tiles bridge the PE from body start to
  the real chain so the HAM clock-gate (1.2 -> 2.4 GHz) is open; any PE
  idle gap resets the busy-window progress (measured cold 585ns vs warm
  ~380ns per N=512 pass).
- All 6 matmuls accumulate into ONE PSUM bank (same-bank back-to-back
  accumulation measured at full pass rate); the epilogue runs the two
  halves in PARALLEL: half 0 y=(acc+bias_u)*(1/I) via DVE tensor_scalar,
  half 1 via ACT Identity (acc*(1/I)+bias_s), each followed by its
  half-store on its own HWDGE queue.
- NO keep-warm dummy stores: a tiny DRAM write has ~3.5us completion
  latency and its completion semaphore sits ahead of the y stores in the
  queue FIFO, delaying the measured end by far more than the ~0.5us
  queue re-startup it was meant to hide (measured both ways).
"""

from contextlib import ExitStack

import numpy as np

import concourse.bass as bass
import concourse.tile as tile
from concourse import bacc, mybir
from concourse.bass_utils import run_bass_kernel_spmd

N_CORES = 8
B, I, O, D = 2048, 256, 256, 4
RB, SO = 4, 2  # batch shards x output shards
BL = B // RB  # 512 batch rows per core
OL = O // SO  # 128 output cols per core
F16 = mybir.dt.float16
F32 = mybir.dt.float32
INV_I = 1.0 / I

_cache = {}


def _build_program():
    nc = bacc.Bacc("TRN2", target_bir_lowering=False, debug=False, num_devices=N_CORES)

    # xt{ih}[i, b] = x[b, ih*128 + i]  (pre-transposed on host)
    xt0_d = nc.dram_tensor("xt0", [128, BL], F16, kind="ExternalInput")
    xt1_d = nc.dram_tensor("xt1", [128, BL], F16, kind="ExternalInput")
    # packed V per ih half: col d*OL + o holds V[d, ih*128+i, o];
    # vb0 additionally carries the unscaled bias in its last column.
    # Two SWDGE descriptors: the ih0 half (which gates the first matmuls)
    # lands ~0.5us earlier than one combined descriptor would; the ih1
    # half's consumers are DVE-gated until ~1us later anyway.
    vb0_d = nc.dram_tensor("vb0", [128, 3 * OL + 1], F16, kind="ExternalInput")
    vb1_d = nc.dram_tensor("vb1", [128, 3 * OL], F16, kind="ExternalInput")
    # transposed fp16 output [o_local, b_local]
    y_d = nc.dram_tensor("y", [OL, BL], F16, kind="ExternalOutput")

    with tile.TileContext(nc) as tc, ExitStack() as ctx:
        pool = ctx.enter_context(tc.tile_pool(name="main", bufs=1))
        psum = ctx.enter_context(
            tc.tile_pool(name="psum", bufs=1, space=bass.MemorySpace.PSUM)
        )

        # PE warmup operands (DVE is idle this early; values are irrelevant)
        wu_w = pool.tile([128, 128], F16, tag="wu_w")
        nc.vector.memset(wu_w[:], 1.0)
        wu_r = pool.tile([128, 512], F16, tag="wu_r")
        nc.vector.memset(wu_r[:], 1.0)

        # Three parallel input paths, one descriptor each.
        xt = {}
        for ih, (eng, xd) in enumerate(((nc.sync, xt0_d), (nc.scalar, xt1_d))):
            xt[ih] = pool.tile([128, BL], F16, tag=f"xt{ih}", name=f"xt{ih}")
            eng.dma_start(xt[ih][:], xd[:])
        vb0 = pool.tile([128, 3 * OL + 1], F16, tag="vb0")
        nc.gpsimd.dma_start(vb0[:], vb0_d[:])
        vb1 = pool.tile([128, 3 * OL], F16, tag="vb1")
        nc.gpsimd.dma_start(vb1[:], vb1_d[:])
        vb = {0: vb0, 1: vb1}

        # Warmup matmul bridge: the HAM clock-gate needs ~3.4us of sustained
        # PE busy (free-running 4096-cycle window) before it opens 2.4 GHz,
        # so start the PE AS EARLY AS POSSIBLE: two tiny N=128 matmuls that
        # need only the first (small) memset, then N=512 ones once wu_r is
        # set. Short sub-us gaps don't re-throttle (going cold needs ~3.4us
        # of idle), so bridge length is not precision-critical.
        wu_acc = psum.tile([128, 512], F32, tag="wu_acc")
        for _ in range(2):
            nc.tensor.matmul(
                wu_acc[:, :128], wu_w[:], wu_w[:], start=True, stop=True
            )
        for _ in range(4):
            nc.tensor.matmul(wu_acc[:], wu_w[:], wu_r[:], start=True, stop=True)
        nc.tensor.matmul(
            wu_acc[:, :256], wu_w[:], wu_r[:, :256], start=True, stop=True
        )

        # tensor_scalar's ptr operand must be fp32: upconvert the fp16 bias
        # column on GpSimd right after vb lands (off the critical path).
        bias_col = pool.tile([128, 1], F32, tag="bias32")
        nc.gpsimd.tensor_scalar(
            bias_col[:],
            vb0[:, 3 * OL : 3 * OL + 1],
            0.0,
            None,
            mybir.AluOpType.add,
        )
        # basis: c = tanh(xT) on ACT (one whole-tile op per half);
        # squares/cubes on DVE, ordered ih0 first (its inputs land first).
        basis = {}
        for ih in range(2):
            c = pool.tile([128, BL], F16, tag=f"c{ih}", name=f"c{ih}")
            nc.scalar.activation(c[:], xt[ih][:], mybir.ActivationFunctionType.Tanh)
            basis[(0, ih)] = c
        for ih in range(2):
            c2 = pool.tile([128, BL], F16, tag=f"c2{ih}", name=f"c2{ih}")
            nc.vector.tensor_mul(c2[:], basis[(0, ih)][:], basis[(0, ih)][:])
            basis[(1, ih)] = c2
            c3 = pool.tile([128, BL], F16, tag=f"c3{ih}", name=f"c3{ih}")
            nc.vector.tensor_mul(c3[:], c2[:], basis[(0, ih)][:])
            basis[(2, ih)] = c3

        # yT[o, b] accumulation: 6 single-pass fp16 matmuls into ONE PSUM
        # bank, ordered by operand readiness.
        acc = psum.tile([128, BL], F32, tag="acc")
        mm_order = [(0, 0), (1, 0), (0, 1), (2, 0), (1, 1), (2, 1)]
        for n, (d, ih) in enumerate(mm_order):
            col = d * OL
            nc.tensor.matmul(
                acc[:OL, :],
                vb[ih][:, col : col + OL],
                basis[(d, ih)][:],
                start=(n == 0),
                stop=(n == len(mm_order) - 1),
            )

        # Fused epilogue, two DVE halves: y = (acc + bias_u) * (1/I) -> fp16
        # SBUF, each half-store on its own HWDGE queue as soon as its half
        # is ready. (A second PSUM reader on another engine serializes
        # behind the first anyway - measured - so one engine loses nothing.)
        hb = BL // 2
        y_sb0 = pool.tile([OL, hb], F16, tag="y_sb0")
        y_sb1 = pool.tile([OL, hb], F16, tag="y_sb1")
        for k, (ysb, dma_eng) in enumerate(((y_sb0, nc.sync), (y_sb1, nc.scalar))):
            nc.vector.tensor_scalar(
                ysb[:],
                acc[:OL, k * hb : (k + 1) * hb],
                bias_col[:],
                INV_I,
                mybir.AluOpType.add,
                mybir.AluOpType.mult,
            )
            dma_eng.dma_start(y_d[:, k * hb : (k + 1) * hb], ysb[:])

    nc.compile()
    return nc


def _get_program():
    if "nc" not in _cache:
        _cache["nc"] = _build_program()
    return _cache["nc"]


def _make_in_maps(x, cheby_coeffs):
    x = np.ascontiguousarray(x, dtype=np.float32)
    W = np.ascontiguousarray(cheby_coeffs, dtype=np.float32)
    assert x.shape == (B, I) and W.shape == (I, O, D)

    V = np.stack(
        [
            W[:, :, 1] - 3.0 * W[:, :, 3],
            2.0 * W[:, :, 2],
            4.0 * W[:, :, 3],
        ]
    ).astype(np.float16)  # [3, I, O] unscaled
    bias_u = (W[:, :, 0] - W[:, :, 2]).sum(axis=0, dtype=np.float32)  # [O] unscaled

    xt_shards = []
    for rb in range(RB):
        xs = x[rb * BL : (rb + 1) * BL, :].T.astype(np.float16)  # [I, BL]
        xt_shards.append(
            (np.ascontiguousarray(xs[:128]), np.ascontiguousarray(xs[128:]))
        )
    vb_shards = []
    for so in range(SO):
        vb0 = np.empty((128, 3 * OL + 1), dtype=np.float16)
        vb1 = np.empty((128, 3 * OL), dtype=np.float16)
        for d in range(3):
            vb0[:, d * OL : (d + 1) * OL] = V[d, :128, so * OL : (so + 1) * OL]
            vb1[:, d * OL : (d + 1) * OL] = V[d, 128:, so * OL : (so + 1) * OL]
        vb0[:, 3 * OL] = bias_u[so * OL : (so + 1) * OL].astype(np.float16)
        vb_shards.append((vb0, vb1))
    in_maps = []
    for c_id in range(N_CORES):
        rb, so = divmod(c_id, SO)
        in_maps.append(
            {
                "xt0": xt_shards[rb][0],
                "xt1": xt_shards[rb][1],
                "vb0": vb_shards[so][0],
                "vb1": vb_shards[so][1],
            }
        )
    return in_maps


def kernel(x, cheby_coeffs):
    nc = _get_program()
    in_maps = _make_in_maps(x, cheby_coeffs)
    res = run_bass_kernel_spmd(nc, in_maps, list(range(N_CORES)))
    y = np.empty((B, O), dtype=np.float32)
    for c_id in range(N_CORES):
        rb, so = divmod(c_id, SO)
        y[rb * BL : (rb + 1) * BL, so * OL : (so + 1) * OL] = (
            res.results[c_id]["y"].T.astype(np.float32)
        )
    return y
